# revision 20
# baseline (speedup 1.0000x reference)
"""Conv2D 3x3 (stride 1, pad 1) NCHW/OIHW, data-parallel over 8 NeuronCores.

Full inputs: x (16,32,224,224) f32, weight (64,32,3,3) f32, bias (64,) f32.
Full output: (16,64,224,224) f32.

Raw-Bass SPMD kernel, per core (2 images), per 28-row block:
  - Host pre-stages x as xst[b, ic, img, s, c] = xpad[img, ic, 28b+s, c]
    (s in 0..29, ic-major) so ONE per-block input DMA writes the im2col
    buffer's group 0 directly (XB[0:32, img, s, c]); its DRAM AP lowers to
    [[13560,32],[1,13560]] (outer dim 32) which HWDGE spreads across all
    16 SDMA engine slots.  A multi-dim gather here would collapse to a
    small outer dim and serialize the input load on a couple of engines.
  - XB[96, 2, 30, 226] holds both images (img is a free dim); partition
    group g holds rows shifted down by g.  DVE builds groups 1/2 as
    shifted copies of group 0 - both images per instruction - in two
    14-row chunks so the PE can start a block after the first chunk.
  - Each output row-pair = 3 PSUM-accumulated matmuls (K=96, M=64, N=448),
    dx realized as a free-dim offset.  The two images ride different PE
    column groups (PSUM partitions 0-63 / 64-127) and overlap in the array.
  - ScalarE evacuates PSUM + bias -> OUT bf16 (both images at once).
  - The sync engine issues all DMAs, ordered by gate-fire time (input of
    block b gated on PE of b-XR; output halves of block k gated on their
    evac), so an output wait never delays a later input issue.
  - y is stored bf16 (halves store traffic); the host upcasts to f32.
    bf16 rounding adds ~1e-3 rel err on top of the bf16-input ~2.5e-3,
    well inside the 2e-2 gate.
  - All cross-engine sync is explicit semaphores; consumers of a DMA
    semaphore wait for its full count.  Single-wait instructions only
    (the local walrus rejects multi-wait).
"""

import sys

sys.path.insert(0, "/opt/trn_rl_repo")

from contextlib import ExitStack

import numpy as np

import concourse.bass as bass
from concourse import mybir
from concourse.bass_utils import run_bass_kernel_spmd

N_CORES = 8
IMGS_PER_CORE = 2
IC, OC, H, W = 32, 64, 224, 224
HP, WP = 226, 226  # padded
BLK = 28  # output rows per block
N_BLK = H // BLK
PPB = BLK // 2  # row-pairs per block (14)
CH = 14  # rows in the first DVE copy chunk (covers pairs 0..CH//2-1)
XR = 5  # XB ring depth
OR = 3  # out ring depth
NPS = 8  # psum banks in rotation

DT_MODE = "bf16"

TRACE = False  # test.py can flip this to get LAST_EXEC_NS
LAST_EXEC_NS = None
LAST_RESULTS = None

_nc_cache = {}


def _install_ntff_shim():
    """The agent image's antenv lacks axon_hooks; recreate the NTFF profile
    hook via ctypes against libaxon_pjrt.so (same ABI trn_boot.py uses)."""
    try:
        import antenv.axon_hooks  # noqa: F401

        return
    except ImportError:
        pass
    import contextlib
    import ctypes
    import types

    so_path = "/opt/axon/libaxon_pjrt.so"
    lib = ctypes.CDLL(so_path)
    if not hasattr(lib, "axon_start_nrt_profile"):
        return
    lib.axon_start_nrt_profile.argtypes = [
        ctypes.POINTER(ctypes.c_int64),
        ctypes.c_size_t,
    ]
    lib.axon_start_nrt_profile.restype = ctypes.c_int64
    lib.axon_stop_nrt_profile.argtypes = [ctypes.c_char_p]
    lib.axon_stop_nrt_profile.restype = ctypes.c_int64

    @contextlib.contextmanager
    def _hook(output_dir, device_ids):
        import jax

        jax.devices()
        if device_ids:
            ids = (ctypes.c_int64 * len(device_ids))(*device_ids)
            rc = lib.axon_start_nrt_profile(ids, len(device_ids))
        else:
            rc = lib.axon_start_nrt_profile(None, 0)
        if rc != 0:
            raise RuntimeError(f"axon_start_nrt_profile rc={rc}")
        try:
            yield
        finally:
            n = lib.axon_stop_nrt_profile(str(output_dir).encode())
            print(f"ntff profile: {n} file(s) written to {output_dir}")

    mod = types.ModuleType("antenv.axon_hooks")
    mod.get_axon_ntff_profile_hook = lambda: _hook
    mod.set_axon_ntff_profile_hook = lambda h: None
    import antenv

    sys.modules["antenv.axon_hooks"] = mod
    antenv.axon_hooks = mod


def _build_nc(mode: str) -> bass.Bass:
    f32 = mybir.dt.float32
    in_dt = mybir.dt.bfloat16 if mode == "bf16" else f32

    nc = bass.Bass()
    xst = nc.dram_tensor(
        "xst", [N_BLK, IC, IMGS_PER_CORE, BLK + 2, WP], in_dt, kind="ExternalInput"
    )
    wt = nc.dram_tensor("wt", [96, 3, OC], in_dt, kind="ExternalInput")
    bias = nc.dram_tensor("bias", [128, 1], f32, kind="ExternalInput")
    y = nc.dram_tensor("y", [IMGS_PER_CORE, OC, H, W], in_dt, kind="ExternalOutput")

    ctx = ExitStack()
    wt_sb = ctx.enter_context(nc.sbuf_tensor("wt_sb", [96, 3, OC], in_dt))
    bias_sb = ctx.enter_context(nc.sbuf_tensor("bias_sb", [128, 1], f32))
    xb = [
        ctx.enter_context(
            nc.sbuf_tensor(f"xb_{r}", [96, IMGS_PER_CORE, BLK + 2, WP], in_dt)
        )
        for r in range(XR)
    ]
    outb = [
        ctx.enter_context(nc.sbuf_tensor(f"outb_{s}", [128, BLK, W], in_dt))
        for s in range(OR)
    ]
    ps = [
        ctx.enter_context(nc.psum_tensor(f"ps_{k}", [128, 2, W], f32))
        for k in range(NPS)
    ]

    s_wt = ctx.enter_context(nc.semaphore("s_wt"))
    s_bias = ctx.enter_context(nc.semaphore("s_bias"))
    s_x = [ctx.enter_context(nc.semaphore(f"s_x_{r}")) for r in range(XR)]
    s_yo = [
        [ctx.enter_context(nc.semaphore(f"s_yo_{s}_{h}")) for h in range(2)]
        for s in range(OR)
    ]
    s_cp = ctx.enter_context(nc.semaphore("s_cp"))
    s_mm = ctx.enter_context(nc.semaphore("s_mm"))
    s_ev = ctx.enter_context(nc.semaphore("s_ev"))

    with ctx, nc.Block() as block:

        @block.sync
        def _(sync):
            # One serial stream, statements ordered by gate-fire time so an
            # output wait never delays a later input issue.  Gates:
            # in(b) <- s_mm = PPB*(b-XR+1); out(k,h1) <- s_ev = PPB*(k+1)
            # (same pair count, slightly later); out(k,h0) <- s_ev =
            # PPB*k + PPB//2.
            yo_count = [[0, 0] for _ in range(OR)]

            def out_dma(k, r0, r1, h):
                i0 = k * BLK
                sync.dma_start(
                    out=y[:, :, i0 + r0 : i0 + r1, :],
                    in_=outb[k % OR][:, r0:r1, :],
                ).then_inc(s_yo[k % OR][h], 16)
                yo_count[k % OR][h] += 1

            for b in range(N_BLK + XR):
                if b == 2:
                    # wt/bias issued after the first two input blocks: their
                    # 96+128 tiny descriptors would otherwise delay block 0's
                    # input (and the whole pipeline ramp)
                    sync.dma_start(out=wt_sb[:, :, :], in_=wt[:, :, :]).then_inc(
                        s_wt, 16
                    )
                    sync.dma_start(out=bias_sb[:, :], in_=bias[:, :]).then_inc(
                        s_bias, 16
                    )
                if b < N_BLK:
                    if b >= XR:
                        # XB slot reuse: PE matmuls of block b-XR done (implies
                        # the DVE shift copies of b-XR are done too)
                        sync.wait_ge(s_mm, PPB * (b - XR + 1))
                    sync.dma_start(out=xb[b % XR][0:32, :, :, :], in_=xst[b]).then_inc(
                        s_x[b % XR], 16
                    )
                if b >= XR:
                    k1 = b - XR
                    if k1 == N_BLK - 1:
                        # split the final store so the drain tail is short
                        sync.wait_ge(s_ev, PPB * k1 + 11)
                        out_dma(k1, 14, 22, 1)
                        sync.wait_ge(s_ev, PPB * (k1 + 1))
                        out_dma(k1, 22, 28, 1)
                    else:
                        sync.wait_ge(s_ev, PPB * (k1 + 1))
                        out_dma(k1, PPB, BLK, 1)
                k0 = b - XR + 1
                if 0 <= k0 < N_BLK:
                    sync.wait_ge(s_ev, PPB * k0 + PPB // 2)
                    out_dma(k0, 0, PPB, 0)
            # kernel completion: all output DMAs drained
            for s in range(OR):
                for h in range(2):
                    sync.wait_ge(s_yo[s][h], 16 * yo_count[s][h])

        @block.vector
        def _(v):
            for b in range(N_BLK):
                r = b % XR
                v.wait_ge(s_x[r], 16 * (b // XR + 1))
                if b >= XR:
                    # g1/g2 overwrite safety: PE done with block b-XR
                    v.wait_ge(s_mm, PPB * (b - XR + 1))
                t = xb[r]
                # chunk A: slots 0:CH (both images per copy)
                v.tensor_copy(
                    out=t[32:64, :, 0:CH, :], in_=t[0:32, :, 1 : CH + 1, :]
                ).then_inc(s_cp, 1)
                v.tensor_copy(
                    out=t[64:96, :, 0:CH, :], in_=t[0:32, :, 2 : CH + 2, :]
                ).then_inc(s_cp, 1)
                # chunk B: slots CH:BLK
                v.tensor_copy(
                    out=t[32:64, :, CH:BLK, :], in_=t[0:32, :, CH + 1 : BLK + 1, :]
                ).then_inc(s_cp, 1)
                v.tensor_copy(
                    out=t[64:96, :, CH:BLK, :], in_=t[0:32, :, CH + 2 : BLK + 2, :]
                ).then_inc(s_cp, 1)

        @block.tensor
        def _(t):
            t.wait_ge(s_wt, 16)
            for b in range(N_BLK):
                r = b % XR
                t.wait_ge(s_x[r], 16 * (b // XR + 1))
                t.wait_ge(s_cp, 4 * b + 2)
                for p in range(PPB):
                    if p == CH // 2:
                        t.wait_ge(s_cp, 4 * b + 4)
                    gp = b * PPB + p
                    if gp >= NPS:
                        t.wait_ge(s_ev, gp - NPS + 1)
                    bank = ps[gp % NPS]
                    b0 = 2 * p
                    last = None
                    for dx in range(3):
                        for img in range(IMGS_PER_CORE):
                            last = nc.tensor.matmul(
                                bank[img * OC : (img + 1) * OC, :, :],
                                wt_sb[:, dx, :],
                                xb[r][:, img, b0 : b0 + 2, dx : dx + W],
                                start=dx == 0,
                                stop=dx == 2,
                                skip_group_check=True,
                            )
                    last.then_inc(s_mm, 1)

        @block.scalar
        def _(sc):
            sc.wait_ge(s_bias, 16)
            for b in range(N_BLK):
                if b >= OR:
                    for h in range(2):
                        sc.wait_ge(s_yo[b % OR][h], 16 * ((b - OR) // OR + 1))
                ob = outb[b % OR]
                for p in range(PPB):
                    gp = b * PPB + p
                    sc.wait_ge(s_mm, gp + 1)
                    sc.activation(
                        ob[:, 2 * p : 2 * p + 2, :],
                        ps[gp % NPS][:, :, :],
                        mybir.ActivationFunctionType.Identity,
                        bias=bias_sb[:, :],
                    ).then_inc(s_ev, 1)

    return nc


def _get_nc(mode: str) -> bass.Bass:
    if mode not in _nc_cache:
        _nc_cache[mode] = _build_nc(mode)
    return _nc_cache[mode]


def kernel(x: np.ndarray, weight: np.ndarray, bias: np.ndarray) -> np.ndarray:
    global LAST_EXEC_NS, LAST_RESULTS
    mode = DT_MODE
    n = x.shape[0]
    assert n == N_CORES * IMGS_PER_CORE

    if mode == "bf16":
        import ml_dtypes

        in_np = ml_dtypes.bfloat16
    else:
        in_np = np.float32

    xp = np.zeros((n, IC, HP, WP), dtype=in_np)
    xp[:, :, 1 : H + 1, 1 : W + 1] = x
    # WT[dy*32+ic, dx, oc] = weight[oc, ic, dy, dx]
    wt = np.ascontiguousarray(weight.transpose(2, 1, 3, 0).reshape(96, 3, OC)).astype(
        in_np
    )
    b2 = np.ascontiguousarray(np.tile(bias.reshape(OC, 1), (2, 1))).astype(np.float32)

    # Stage to xst[core, b, ic, img, s, c] = xpad[img, ic, BLK*b + s, c]
    si, sc, sr, scol = xp.strides
    v = np.lib.stride_tricks.as_strided(
        xp,
        shape=(N_CORES, IMGS_PER_CORE, IC, N_BLK, BLK + 2, WP),
        strides=(si * IMGS_PER_CORE, si, sc, BLK * sr, sr, scol),
    )
    # -> [core, b, ic, img, s, c]
    xst = np.ascontiguousarray(v.transpose(0, 3, 2, 1, 4, 5))

    nc = _get_nc(mode)
    in_maps = [{"xst": xst[i], "wt": wt, "bias": b2} for i in range(N_CORES)]
    if TRACE:
        _install_ntff_shim()
    res = run_bass_kernel_spmd(nc, in_maps, core_ids=list(range(N_CORES)), trace=TRACE)
    LAST_EXEC_NS = res.exec_time_ns
    LAST_RESULTS = res
    y = np.concatenate([r["y"] for r in res.results], axis=0)
    return y.astype(np.float32)


# revision 38
# speedup vs baseline: 1.1601x; 1.1601x over previous
"""Conv2D 3x3 (stride 1, pad 1) NCHW/OIHW, data-parallel over 8 NeuronCores.

Full inputs: x (16,32,224,224) f32, weight (64,32,3,3) f32, bias (64,) f32.
Full output: (16,64,224,224) f32.

Raw-Bass SPMD kernel, per core (2 images), per 28-row block:
  - Host pre-stages x as xst[b, ic, img, s, c] = xpad[img, ic, 28b+s, c]
    (s in 0..29, ic-major) so ONE per-block input DMA writes the im2col
    buffer's group 0 directly (XB[0:32, img, s, c]); its DRAM AP lowers to
    [[13560,32],[1,13560]] (outer dim 32) which HWDGE spreads across all
    16 SDMA engine slots.  A multi-dim gather here would collapse to a
    small outer dim and serialize the input load on a couple of engines.
  - XB[96, 2, 30, 226] holds both images (img is a free dim); partition
    group g holds rows shifted down by g.  DVE builds groups 1/2 as
    shifted copies of group 0 - both images per instruction - in two
    14-row chunks so the PE can start a block after the first chunk.
  - Each output row-pair = 3 PSUM-accumulated matmuls (K=96, M=64, N=448),
    dx realized as a free-dim offset.  The two images ride different PE
    column groups (PSUM partitions 0-63 / 64-127) and overlap in the array.
  - ScalarE evacuates PSUM + bias -> OUT bf16 (both images at once).
  - The sync engine issues all DMAs, ordered by gate-fire time (input of
    block b gated on PE of b-XR; output halves of block k gated on their
    evac), so an output wait never delays a later input issue.
  - y is stored bf16 (halves store traffic); the host upcasts to f32.
    bf16 rounding adds ~1e-3 rel err on top of the bf16-input ~2.5e-3,
    well inside the 2e-2 gate.
  - All cross-engine sync is explicit semaphores; consumers of a DMA
    semaphore wait for its full count.  Single-wait instructions only
    (the local walrus rejects multi-wait).
"""

import sys

sys.path.insert(0, "/opt/trn_rl_repo")

from contextlib import ExitStack

import numpy as np

import concourse.bass as bass
from concourse import mybir
from concourse.bass_utils import run_bass_kernel_spmd

N_CORES = 8
IMGS_PER_CORE = 2
IC, OC, H, W = 32, 64, 224, 224
HP, WP = 226, 226  # padded
BLK = 28  # output rows per block
N_BLK = H // BLK
PPB = BLK // 2  # row-pairs per block (14)
CH = 8  # rows in the first DVE copy chunk (covers pairs 0..CH//2-1)
XR = 5  # XB ring depth
OR = 3  # out ring depth
NPS = 8  # psum banks in rotation

DT_MODE = "bf16"
USE_XG = False  # block-0 g1/g2 via host-staged DMA instead of DVE copies

TRACE = False  # test.py can flip this to get LAST_EXEC_NS
LAST_EXEC_NS = None
LAST_RESULTS = None

_nc_cache = {}


def _install_ntff_shim():
    """The agent image's antenv lacks axon_hooks; recreate the NTFF profile
    hook via ctypes against libaxon_pjrt.so (same ABI trn_boot.py uses)."""
    try:
        import antenv.axon_hooks  # noqa: F401

        return
    except ImportError:
        pass
    import contextlib
    import ctypes
    import types

    so_path = "/opt/axon/libaxon_pjrt.so"
    lib = ctypes.CDLL(so_path)
    if not hasattr(lib, "axon_start_nrt_profile"):
        return
    lib.axon_start_nrt_profile.argtypes = [
        ctypes.POINTER(ctypes.c_int64),
        ctypes.c_size_t,
    ]
    lib.axon_start_nrt_profile.restype = ctypes.c_int64
    lib.axon_stop_nrt_profile.argtypes = [ctypes.c_char_p]
    lib.axon_stop_nrt_profile.restype = ctypes.c_int64

    @contextlib.contextmanager
    def _hook(output_dir, device_ids):
        import jax

        jax.devices()
        if device_ids:
            ids = (ctypes.c_int64 * len(device_ids))(*device_ids)
            rc = lib.axon_start_nrt_profile(ids, len(device_ids))
        else:
            rc = lib.axon_start_nrt_profile(None, 0)
        if rc != 0:
            raise RuntimeError(f"axon_start_nrt_profile rc={rc}")
        try:
            yield
        finally:
            n = lib.axon_stop_nrt_profile(str(output_dir).encode())
            print(f"ntff profile: {n} file(s) written to {output_dir}")

    mod = types.ModuleType("antenv.axon_hooks")
    mod.get_axon_ntff_profile_hook = lambda: _hook
    mod.set_axon_ntff_profile_hook = lambda h: None
    import antenv

    sys.modules["antenv.axon_hooks"] = mod
    antenv.axon_hooks = mod


def _build_nc(mode: str) -> bass.Bass:
    f32 = mybir.dt.float32
    in_dt = mybir.dt.bfloat16 if mode == "bf16" else f32
    CP0 = 2 if USE_XG else 4  # DVE copies in block 0

    nc = bass.Bass()
    xst = nc.dram_tensor(
        "xst", [N_BLK, IC, IMGS_PER_CORE, BLK + 2, WP], in_dt, kind="ExternalInput"
    )
    # block-0 ramp accelerator: groups 1/2 for slots 0:CH, host-staged, so
    # the PE's first pairs need no DVE copy at all
    xg = nc.dram_tensor(
        "xg", [2, IC, IMGS_PER_CORE, CH, WP], in_dt, kind="ExternalInput"
    )
    wt = nc.dram_tensor("wt", [96, 3, OC], in_dt, kind="ExternalInput")
    bias = nc.dram_tensor("bias", [128, 1], f32, kind="ExternalInput")
    y = nc.dram_tensor("y", [IMGS_PER_CORE, OC, H, W], in_dt, kind="ExternalOutput")

    ctx = ExitStack()
    wt_sb = ctx.enter_context(nc.sbuf_tensor("wt_sb", [96, 3, OC], in_dt))
    bias_sb = ctx.enter_context(nc.sbuf_tensor("bias_sb", [128, 1], f32))
    xb = [
        ctx.enter_context(
            nc.sbuf_tensor(f"xb_{r}", [96, IMGS_PER_CORE, BLK + 2, WP], in_dt)
        )
        for r in range(XR)
    ]
    outb = [
        ctx.enter_context(nc.sbuf_tensor(f"outb_{s}", [128, BLK, W], in_dt))
        for s in range(OR)
    ]
    ps = [
        ctx.enter_context(nc.psum_tensor(f"ps_{k}", [128, 2, W], f32))
        for k in range(NPS)
    ]

    s_wt = ctx.enter_context(nc.semaphore("s_wt"))
    s_bias = ctx.enter_context(nc.semaphore("s_bias"))
    s_x = [ctx.enter_context(nc.semaphore(f"s_x_{r}")) for r in range(XR)]
    s_x0 = ctx.enter_context(nc.semaphore("s_x0"))
    s_xg = ctx.enter_context(nc.semaphore("s_xg"))
    s_yo = [
        [ctx.enter_context(nc.semaphore(f"s_yo_{s}_{h}")) for h in range(2)]
        for s in range(OR)
    ]
    s_cp = ctx.enter_context(nc.semaphore("s_cp"))
    s_mm = ctx.enter_context(nc.semaphore("s_mm"))
    s_ev = ctx.enter_context(nc.semaphore("s_ev"))

    with ctx, nc.Block() as block:

        @block.sync
        def _(sync):
            # One serial stream, statements ordered by gate-fire time so an
            # output wait never delays a later input issue.  Gates:
            # in(b) <- s_mm = PPB*(b-XR+1); out(k,h1) <- s_ev = PPB*(k+1)
            # (same pair count, slightly later); out(k,h0) <- s_ev =
            # PPB*k + PPB//2.
            yo_count = [[0, 0] for _ in range(OR)]

            def out_dma(k, r0, r1, h):
                i0 = k * BLK
                sync.dma_start(
                    out=y[:, :, i0 + r0 : i0 + r1, :],
                    in_=outb[k % OR][:, r0:r1, :],
                ).then_inc(s_yo[k % OR][h], 16)
                yo_count[k % OR][h] += 1

            for b in range(N_BLK + XR):
                if b == 2:
                    # wt/bias issued after the first two input blocks: their
                    # 96+128 tiny descriptors would otherwise delay block 0's
                    # input (and the whole pipeline ramp)
                    sync.dma_start(out=wt_sb[:, :, :], in_=wt[:, :, :]).then_inc(
                        s_wt, 16
                    )
                    sync.dma_start(out=bias_sb[:, :], in_=bias[:, :]).then_inc(
                        s_bias, 16
                    )
                if b < N_BLK:
                    if b >= XR:
                        # XB slot reuse: PE matmuls of block b-XR done (implies
                        # the DVE shift copies of b-XR are done too)
                        sync.wait_ge(s_mm, PPB * (b - XR + 1))
                    if b == 0:
                        # block 0 only: land groups 1/2 (host-staged) and
                        # rows 0:CH+2 of group 0 first, each on its own
                        # semaphore, so the PE can start with no DVE work
                        if USE_XG:
                            sync.dma_start(
                                out=xb[0][32:96, :, 0:CH, :], in_=xg[:, :, :, :, :]
                            ).then_inc(s_xg, 16)
                        sync.dma_start(
                            out=xb[0][0:32, :, 0 : CH + 2, :],
                            in_=xst[0][:, :, 0 : CH + 2, :],
                        ).then_inc(s_x0, 16)
                        sync.dma_start(
                            out=xb[0][0:32, :, CH + 2 : BLK + 2, :],
                            in_=xst[0][:, :, CH + 2 : BLK + 2, :],
                        ).then_inc(s_x[0], 16)
                    else:
                        sync.dma_start(
                            out=xb[b % XR][0:32, :, :, :], in_=xst[b]
                        ).then_inc(s_x[b % XR], 16)
                if b >= XR:
                    k1 = b - XR
                    if k1 == N_BLK - 1:
                        # split the final store so the drain tail is short
                        sync.wait_ge(s_ev, PPB * k1 + 11)
                        out_dma(k1, 14, 22, 1)
                        sync.wait_ge(s_ev, PPB * (k1 + 1))
                        out_dma(k1, 22, 28, 1)
                    else:
                        sync.wait_ge(s_ev, PPB * (k1 + 1))
                        out_dma(k1, PPB, BLK, 1)
                k0 = b - XR + 1
                if 0 <= k0 < N_BLK:
                    sync.wait_ge(s_ev, PPB * k0 + PPB // 2)
                    out_dma(k0, 0, PPB, 0)
            # kernel completion: all output DMAs drained
            for s in range(OR):
                for h in range(2):
                    sync.wait_ge(s_yo[s][h], 16 * yo_count[s][h])

        @block.vector
        def _(v):
            for b in range(N_BLK):
                r = b % XR
                t = xb[r]
                if b == 0:
                    # chunk A arrives by DMA (xg); only chunk B is copied
                    v.wait_ge(s_x0, 16)
                    if not USE_XG:
                        v.tensor_copy(
                            out=t[32:64, :, 0:CH, :], in_=t[0:32, :, 1 : CH + 1, :]
                        ).then_inc(s_cp, 1)
                        v.tensor_copy(
                            out=t[64:96, :, 0:CH, :], in_=t[0:32, :, 2 : CH + 2, :]
                        ).then_inc(s_cp, 1)
                    v.wait_ge(s_x[0], 16)
                else:
                    v.wait_ge(s_x[r], 16 * (b // XR + 1))
                    if b >= XR:
                        # g1/g2 overwrite safety: PE done with block b-XR
                        v.wait_ge(s_mm, PPB * (b - XR + 1))
                    # chunk A: slots 0:CH (both images per copy)
                    v.tensor_copy(
                        out=t[32:64, :, 0:CH, :], in_=t[0:32, :, 1 : CH + 1, :]
                    ).then_inc(s_cp, 1)
                    v.tensor_copy(
                        out=t[64:96, :, 0:CH, :], in_=t[0:32, :, 2 : CH + 2, :]
                    ).then_inc(s_cp, 1)
                # chunk B: slots CH:BLK
                v.tensor_copy(
                    out=t[32:64, :, CH:BLK, :], in_=t[0:32, :, CH + 1 : BLK + 1, :]
                ).then_inc(s_cp, 1)
                v.tensor_copy(
                    out=t[64:96, :, CH:BLK, :], in_=t[0:32, :, CH + 2 : BLK + 2, :]
                ).then_inc(s_cp, 1)

        @block.tensor
        def _(t):
            t.wait_ge(s_wt, 16)
            for b in range(N_BLK):
                r = b % XR
                if b == 0:
                    # pairs 0..CH//2-1: g0 from sub-DMA 1, g1/g2 from xg
                    t.wait_ge(s_x0, 16)
                    if USE_XG:
                        t.wait_ge(s_xg, 16)
                    else:
                        t.wait_ge(s_cp, 2)
                else:
                    t.wait_ge(s_x[r], 16 * (b // XR + 1))
                    t.wait_ge(s_cp, 4 * b + CP0 - 2)
                for p in range(PPB):
                    if p == CH // 2:
                        # chunk B copies done (implies input fully landed)
                        t.wait_ge(s_cp, 4 * b + CP0)
                    gp = b * PPB + p
                    if gp >= NPS:
                        t.wait_ge(s_ev, gp - NPS + 1)
                    bank = ps[gp % NPS]
                    b0 = 2 * p
                    last = None
                    for dx in range(3):
                        for img in range(IMGS_PER_CORE):
                            last = nc.tensor.matmul(
                                bank[img * OC : (img + 1) * OC, :, :],
                                wt_sb[:, dx, :],
                                xb[r][:, img, b0 : b0 + 2, dx : dx + W],
                                start=dx == 0,
                                stop=dx == 2,
                                skip_group_check=True,
                            )
                    last.then_inc(s_mm, 1)

        @block.scalar
        def _(sc):
            sc.wait_ge(s_bias, 16)
            for b in range(N_BLK):
                if b >= OR:
                    for h in range(2):
                        sc.wait_ge(s_yo[b % OR][h], 16 * ((b - OR) // OR + 1))
                ob = outb[b % OR]
                for p in range(PPB):
                    gp = b * PPB + p
                    sc.wait_ge(s_mm, gp + 1)
                    sc.activation(
                        ob[:, 2 * p : 2 * p + 2, :],
                        ps[gp % NPS][:, :, :],
                        mybir.ActivationFunctionType.Identity,
                        bias=bias_sb[:, :],
                    ).then_inc(s_ev, 1)

    return nc


def _get_nc(mode: str) -> bass.Bass:
    key = (mode, USE_XG)
    if key not in _nc_cache:
        _nc_cache[key] = _build_nc(mode)
    return _nc_cache[key]


def kernel(x: np.ndarray, weight: np.ndarray, bias: np.ndarray) -> np.ndarray:
    global LAST_EXEC_NS, LAST_RESULTS
    mode = DT_MODE
    n = x.shape[0]
    assert n == N_CORES * IMGS_PER_CORE

    if mode == "bf16":
        import ml_dtypes

        in_np = ml_dtypes.bfloat16
    else:
        in_np = np.float32

    xp = np.zeros((n, IC, HP, WP), dtype=in_np)
    xp[:, :, 1 : H + 1, 1 : W + 1] = x
    # WT[dy*32+ic, dx, oc] = weight[oc, ic, dy, dx]
    wt = np.ascontiguousarray(weight.transpose(2, 1, 3, 0).reshape(96, 3, OC)).astype(
        in_np
    )
    b2 = np.ascontiguousarray(np.tile(bias.reshape(OC, 1), (2, 1))).astype(np.float32)

    # Stage to xst[core, b, ic, img, s, c] = xpad[img, ic, BLK*b + s, c]
    si, sc, sr, scol = xp.strides
    v = np.lib.stride_tricks.as_strided(
        xp,
        shape=(N_CORES, IMGS_PER_CORE, IC, N_BLK, BLK + 2, WP),
        strides=(si * IMGS_PER_CORE, si, sc, BLK * sr, sr, scol),
    )
    # -> [core, b, ic, img, s, c]
    xst = np.ascontiguousarray(v.transpose(0, 3, 2, 1, 4, 5))

    # xg[core, g-1, ic, img, s, c] = xpad[img, ic, s+g, c]  (block-0 ramp)
    xg = np.empty((N_CORES, 2, IC, IMGS_PER_CORE, CH, WP), dtype=in_np)
    for g in (1, 2):
        blk = xp[:, :, g : g + CH, :].reshape(N_CORES, IMGS_PER_CORE, IC, CH, WP)
        xg[:, g - 1] = blk.transpose(0, 2, 1, 3, 4)

    nc = _get_nc(mode)
    in_maps = [
        {"xst": xst[i], "xg": np.ascontiguousarray(xg[i]), "wt": wt, "bias": b2}
        for i in range(N_CORES)
    ]
    if TRACE:
        _install_ntff_shim()
    res = run_bass_kernel_spmd(nc, in_maps, core_ids=list(range(N_CORES)), trace=TRACE)
    LAST_EXEC_NS = res.exec_time_ns
    LAST_RESULTS = res
    y = np.concatenate([r["y"] for r in res.results], axis=0)
    return y.astype(np.float32)


# revision 43
# speedup vs baseline: 1.2423x; 1.0708x over previous
"""Conv2D 3x3 (stride 1, pad 1) NCHW/OIHW, data-parallel over 8 NeuronCores.

Full inputs: x (16,32,224,224) f32, weight (64,32,3,3) f32, bias (64,) f32.
Full output: (16,64,224,224) f32.

Raw-Bass SPMD kernel, per core (2 images), per 28-row block:
  - Host pre-stages x as xst[b, ic, img, s, c] = xpad[img, ic, 28b+s, c]
    (s in 0..29, ic-major) so ONE per-block input DMA writes the im2col
    buffer's group 0 directly (XB[0:32, img, s, c]); its DRAM AP lowers to
    [[13560,32],[1,13560]] (outer dim 32) which HWDGE spreads across all
    16 SDMA engine slots.  A multi-dim gather here would collapse to a
    small outer dim and serialize the input load on a couple of engines.
  - XB[96, 2, 30, 226] holds both images (img is a free dim); partition
    group g holds rows shifted down by g.  DVE builds groups 1/2 as
    shifted copies of group 0 - both images per instruction - in two
    14-row chunks so the PE can start a block after the first chunk.
  - Each output row-pair = 3 PSUM-accumulated matmuls (K=96, M=64, N=448),
    dx realized as a free-dim offset.  The two images ride different PE
    column groups (PSUM partitions 0-63 / 64-127) and overlap in the array.
  - ScalarE evacuates PSUM + bias -> OUT bf16 (both images at once).
  - The sync engine issues all DMAs, ordered by gate-fire time (input of
    block b gated on PE of b-XR; output halves of block k gated on their
    evac), so an output wait never delays a later input issue.
  - y is stored bf16 (halves store traffic); the host upcasts to f32.
    bf16 rounding adds ~1e-3 rel err on top of the bf16-input ~2.5e-3,
    well inside the 2e-2 gate.
  - All cross-engine sync is explicit semaphores; consumers of a DMA
    semaphore wait for its full count.  Single-wait instructions only
    (the local walrus rejects multi-wait).
"""

import sys

sys.path.insert(0, "/opt/trn_rl_repo")

from contextlib import ExitStack

import numpy as np

import concourse.bass as bass
from concourse import mybir
from concourse.bass_utils import run_bass_kernel_spmd

N_CORES = 8
IMGS_PER_CORE = 2
IC, OC, H, W = 32, 64, 224, 224
HP, WP = 226, 226  # padded
BLK = 28  # output rows per block
N_BLK = H // BLK
PPB = BLK // 2  # row-pairs per block (14)
CH = 8  # rows in the first DVE copy chunk (covers pairs 0..CH//2-1)
CH2 = 18  # end row of the second copy chunk
XR = 5  # XB ring depth
OR = 3  # out ring depth
NPS = 8  # psum banks in rotation

DT_MODE = "bf16"
USE_XG = False  # block-0 g1/g2 via host-staged DMA instead of DVE copies

TRACE = False  # test.py can flip this to get LAST_EXEC_NS
LAST_EXEC_NS = None
LAST_RESULTS = None

_nc_cache = {}


def _install_ntff_shim():
    """The agent image's antenv lacks axon_hooks; recreate the NTFF profile
    hook via ctypes against libaxon_pjrt.so (same ABI trn_boot.py uses)."""
    try:
        import antenv.axon_hooks  # noqa: F401

        return
    except ImportError:
        pass
    import contextlib
    import ctypes
    import types

    so_path = "/opt/axon/libaxon_pjrt.so"
    lib = ctypes.CDLL(so_path)
    if not hasattr(lib, "axon_start_nrt_profile"):
        return
    lib.axon_start_nrt_profile.argtypes = [
        ctypes.POINTER(ctypes.c_int64),
        ctypes.c_size_t,
    ]
    lib.axon_start_nrt_profile.restype = ctypes.c_int64
    lib.axon_stop_nrt_profile.argtypes = [ctypes.c_char_p]
    lib.axon_stop_nrt_profile.restype = ctypes.c_int64

    @contextlib.contextmanager
    def _hook(output_dir, device_ids):
        import jax

        jax.devices()
        if device_ids:
            ids = (ctypes.c_int64 * len(device_ids))(*device_ids)
            rc = lib.axon_start_nrt_profile(ids, len(device_ids))
        else:
            rc = lib.axon_start_nrt_profile(None, 0)
        if rc != 0:
            raise RuntimeError(f"axon_start_nrt_profile rc={rc}")
        try:
            yield
        finally:
            n = lib.axon_stop_nrt_profile(str(output_dir).encode())
            print(f"ntff profile: {n} file(s) written to {output_dir}")

    mod = types.ModuleType("antenv.axon_hooks")
    mod.get_axon_ntff_profile_hook = lambda: _hook
    mod.set_axon_ntff_profile_hook = lambda h: None
    import antenv

    sys.modules["antenv.axon_hooks"] = mod
    antenv.axon_hooks = mod


def _build_nc(mode: str) -> bass.Bass:
    f32 = mybir.dt.float32
    in_dt = mybir.dt.bfloat16 if mode == "bf16" else f32
    CP0 = 2 if USE_XG else 4  # DVE copies in block 0

    nc = bass.Bass()
    xst = nc.dram_tensor(
        "xst", [N_BLK, IC, IMGS_PER_CORE, BLK + 2, WP], in_dt, kind="ExternalInput"
    )
    # block-0 ramp accelerator: groups 1/2 for slots 0:CH, host-staged, so
    # the PE's first pairs need no DVE copy at all
    xg = nc.dram_tensor(
        "xg", [2, IC, IMGS_PER_CORE, CH, WP], in_dt, kind="ExternalInput"
    )
    wt = nc.dram_tensor("wt", [96, 3, OC], in_dt, kind="ExternalInput")
    bias = nc.dram_tensor("bias", [128, 1], f32, kind="ExternalInput")
    y = nc.dram_tensor("y", [IMGS_PER_CORE, OC, H, W], in_dt, kind="ExternalOutput")

    ctx = ExitStack()
    wt_sb = ctx.enter_context(nc.sbuf_tensor("wt_sb", [96, 3, OC], in_dt))
    bias_sb = ctx.enter_context(nc.sbuf_tensor("bias_sb", [128, 1], f32))
    xb = [
        ctx.enter_context(
            nc.sbuf_tensor(f"xb_{r}", [96, IMGS_PER_CORE, BLK + 2, WP], in_dt)
        )
        for r in range(XR)
    ]
    outb = [
        ctx.enter_context(nc.sbuf_tensor(f"outb_{s}", [128, BLK, W], in_dt))
        for s in range(OR)
    ]
    ps = [
        ctx.enter_context(nc.psum_tensor(f"ps_{k}", [128, 2, W], f32))
        for k in range(NPS)
    ]

    s_wt = ctx.enter_context(nc.semaphore("s_wt"))
    s_bias = ctx.enter_context(nc.semaphore("s_bias"))
    s_x = [ctx.enter_context(nc.semaphore(f"s_x_{r}")) for r in range(XR)]
    s_x0 = ctx.enter_context(nc.semaphore("s_x0"))
    s_xg = ctx.enter_context(nc.semaphore("s_xg"))
    s_yo = [
        [ctx.enter_context(nc.semaphore(f"s_yo_{s}_{h}")) for h in range(2)]
        for s in range(OR)
    ]
    s_cp = ctx.enter_context(nc.semaphore("s_cp"))
    s_mm = ctx.enter_context(nc.semaphore("s_mm"))
    s_ev = ctx.enter_context(nc.semaphore("s_ev"))

    with ctx, nc.Block() as block:

        @block.sync
        def _(sync):
            # One serial stream, statements ordered by gate-fire time so an
            # output wait never delays a later input issue.  Gates:
            # in(b) <- s_mm = PPB*(b-XR+1); out(k,h1) <- s_ev = PPB*(k+1)
            # (same pair count, slightly later); out(k,h0) <- s_ev =
            # PPB*k + PPB//2.
            yo_count = [[0, 0] for _ in range(OR)]

            def out_dma(k, r0, r1, h):
                i0 = k * BLK
                sync.dma_start(
                    out=y[:, :, i0 + r0 : i0 + r1, :],
                    in_=outb[k % OR][:, r0:r1, :],
                ).then_inc(s_yo[k % OR][h], 16)
                yo_count[k % OR][h] += 1

            for b in range(N_BLK + XR):
                if b < N_BLK:
                    if b >= XR:
                        # XB slot reuse: PE matmuls of block b-XR done (implies
                        # the DVE shift copies of b-XR are done too)
                        sync.wait_ge(s_mm, PPB * (b - XR + 1))
                    if b == 0:
                        # block 0 only: land groups 1/2 (host-staged) and
                        # rows 0:CH+2 of group 0 first, each on its own
                        # semaphore, so the PE can start with no DVE work
                        if USE_XG:
                            sync.dma_start(
                                out=xb[0][32:96, :, 0:CH, :], in_=xg[:, :, :, :, :]
                            ).then_inc(s_xg, 16)
                        sync.dma_start(
                            out=xb[0][0:32, :, 0 : CH + 2, :],
                            in_=xst[0][:, :, 0 : CH + 2, :],
                        ).then_inc(s_x0, 16)
                        sync.dma_start(
                            out=xb[0][0:32, :, CH + 2 : BLK + 2, :],
                            in_=xst[0][:, :, CH + 2 : BLK + 2, :],
                        ).then_inc(s_x[0], 16)
                    else:
                        sync.dma_start(
                            out=xb[b % XR][0:32, :, :, :], in_=xst[b]
                        ).then_inc(s_x[b % XR], 16)
                if b >= XR:
                    k1 = b - XR
                    if k1 == N_BLK - 1:
                        # split the final store so the drain tail is short
                        sync.wait_ge(s_ev, PPB * k1 + 11)
                        out_dma(k1, 14, 22, 1)
                        sync.wait_ge(s_ev, PPB * (k1 + 1))
                        out_dma(k1, 22, 28, 1)
                    else:
                        sync.wait_ge(s_ev, PPB * (k1 + 1))
                        out_dma(k1, PPB, BLK, 1)
                k0 = b - XR + 1
                if 0 <= k0 < N_BLK:
                    sync.wait_ge(s_ev, PPB * k0 + PPB // 2)
                    out_dma(k0, 0, PPB, 0)
            # kernel completion: all output DMAs drained
            for s in range(OR):
                for h in range(2):
                    sync.wait_ge(s_yo[s][h], 16 * yo_count[s][h])

        @block.vector
        def _(v):
            for b in range(N_BLK):
                r = b % XR
                t = xb[r]
                if b == 0:
                    v.wait_ge(s_x0, 16)
                else:
                    v.wait_ge(s_x[r], 16 * (b // XR + 1))
                    if b >= XR:
                        # g1/g2 overwrite safety: PE done with block b-XR
                        v.wait_ge(s_mm, PPB * (b - XR + 1))
                # three chunks (slots 0:CH, CH:CH2, CH2:BLK), each copying
                # groups 1/2 for both images, so the PE's per-pair gates
                # track copy progress finely and never bubble on the ramp
                for c0, c1 in ((0, CH), (CH, CH2), (CH2, BLK)):
                    if b == 0 and c0 == CH:
                        # chunks B/C read group-0 rows from the second
                        # block-0 sub-DMA
                        v.wait_ge(s_x[0], 16)
                    v.tensor_copy(
                        out=t[32:64, :, c0:c1, :], in_=t[0:32, :, c0 + 1 : c1 + 1, :]
                    ).then_inc(s_cp, 1)
                    v.tensor_copy(
                        out=t[64:96, :, c0:c1, :], in_=t[0:32, :, c0 + 2 : c1 + 2, :]
                    ).then_inc(s_cp, 1)

        @block.tensor
        def _(t):
            t.wait_ge(s_wt, 16)
            for b in range(N_BLK):
                r = b % XR
                if b == 0:
                    t.wait_ge(s_x0, 16)
                    t.wait_ge(s_cp, 2)
                else:
                    t.wait_ge(s_x[r], 16 * (b // XR + 1))
                    t.wait_ge(s_cp, 6 * b + 2)
                for p in range(PPB):
                    if p == CH // 2:
                        # chunk B copies done (implies sub-DMA 2 landed)
                        t.wait_ge(s_cp, 6 * b + 4)
                    elif p == CH2 // 2:
                        t.wait_ge(s_cp, 6 * b + 6)
                    gp = b * PPB + p
                    if gp >= NPS:
                        t.wait_ge(s_ev, gp - NPS + 1)
                    bank = ps[gp % NPS]
                    b0 = 2 * p
                    last = None
                    for dx in range(3):
                        for img in range(IMGS_PER_CORE):
                            last = nc.tensor.matmul(
                                bank[img * OC : (img + 1) * OC, :, :],
                                wt_sb[:, dx, :],
                                xb[r][:, img, b0 : b0 + 2, dx : dx + W],
                                start=dx == 0,
                                stop=dx == 2,
                                skip_group_check=True,
                            )
                    last.then_inc(s_mm, 1)

        @block.scalar
        def _(sc):
            # wt/bias ride the Activation engine's own HWDGE ring (qAct),
            # which is empty at kernel start - on the sync ring their 224
            # tiny descriptors would queue behind ~2MB of input and gate
            # the PE's first matmul ~6us late
            sc.dma_start(out=wt_sb[:, :, :], in_=wt[:, :, :]).then_inc(s_wt, 16)
            sc.dma_start(out=bias_sb[:, :], in_=bias[:, :]).then_inc(s_bias, 16)
            sc.wait_ge(s_bias, 16)
            for b in range(N_BLK):
                if b >= OR:
                    for h in range(2):
                        sc.wait_ge(s_yo[b % OR][h], 16 * ((b - OR) // OR + 1))
                ob = outb[b % OR]
                for p in range(PPB):
                    gp = b * PPB + p
                    sc.wait_ge(s_mm, gp + 1)
                    sc.activation(
                        ob[:, 2 * p : 2 * p + 2, :],
                        ps[gp % NPS][:, :, :],
                        mybir.ActivationFunctionType.Identity,
                        bias=bias_sb[:, :],
                    ).then_inc(s_ev, 1)

    return nc


def _get_nc(mode: str) -> bass.Bass:
    key = (mode, USE_XG)
    if key not in _nc_cache:
        _nc_cache[key] = _build_nc(mode)
    return _nc_cache[key]


def kernel(x: np.ndarray, weight: np.ndarray, bias: np.ndarray) -> np.ndarray:
    global LAST_EXEC_NS, LAST_RESULTS
    mode = DT_MODE
    n = x.shape[0]
    assert n == N_CORES * IMGS_PER_CORE

    if mode == "bf16":
        import ml_dtypes

        in_np = ml_dtypes.bfloat16
    else:
        in_np = np.float32

    xp = np.zeros((n, IC, HP, WP), dtype=in_np)
    xp[:, :, 1 : H + 1, 1 : W + 1] = x
    # WT[dy*32+ic, dx, oc] = weight[oc, ic, dy, dx]
    wt = np.ascontiguousarray(weight.transpose(2, 1, 3, 0).reshape(96, 3, OC)).astype(
        in_np
    )
    b2 = np.ascontiguousarray(np.tile(bias.reshape(OC, 1), (2, 1))).astype(np.float32)

    # Stage to xst[core, b, ic, img, s, c] = xpad[img, ic, BLK*b + s, c]
    si, sc, sr, scol = xp.strides
    v = np.lib.stride_tricks.as_strided(
        xp,
        shape=(N_CORES, IMGS_PER_CORE, IC, N_BLK, BLK + 2, WP),
        strides=(si * IMGS_PER_CORE, si, sc, BLK * sr, sr, scol),
    )
    # -> [core, b, ic, img, s, c]
    xst = np.ascontiguousarray(v.transpose(0, 3, 2, 1, 4, 5))

    # xg[core, g-1, ic, img, s, c] = xpad[img, ic, s+g, c]  (block-0 ramp)
    xg = np.empty((N_CORES, 2, IC, IMGS_PER_CORE, CH, WP), dtype=in_np)
    for g in (1, 2):
        blk = xp[:, :, g : g + CH, :].reshape(N_CORES, IMGS_PER_CORE, IC, CH, WP)
        xg[:, g - 1] = blk.transpose(0, 2, 1, 3, 4)

    nc = _get_nc(mode)
    in_maps = [
        {"xst": xst[i], "xg": np.ascontiguousarray(xg[i]), "wt": wt, "bias": b2}
        for i in range(N_CORES)
    ]
    if TRACE:
        _install_ntff_shim()
    res = run_bass_kernel_spmd(nc, in_maps, core_ids=list(range(N_CORES)), trace=TRACE)
    LAST_EXEC_NS = res.exec_time_ns
    LAST_RESULTS = res
    y = np.concatenate([r["y"] for r in res.results], axis=0)
    return y.astype(np.float32)
